# revision 43
# baseline (speedup 1.0000x reference)
"""Trainium2 Bass kernel for the nn_Points problem.

Renders N=1024 anisotropic "diamond" points onto a 3x256x384 canvas:
    t = (pixel - loc) @ M_n          (2-vector per pixel per point)
    mapped = relu(1 - (|t0|+|t1|)/2)
    canvas = sigmoid(4 * sum_n mapped * color_n)

Design (8 NeuronCores, full inputs in / full output out):
  * u = t0+t1, v = t0-t1, so |t0|+|t1| = max(|u|,|v|) and
    mapped'' := min(max(|u|,|v|), 2) - 2 = -2*mapped (colors pre-scaled
    by -c/2 make the canvas matmul come out right).
  * The image is cut into 384 16x16 sub-tiles; an exact SAT cull (tile
    rect vs the preimage of the |t|_1<=2 diamond) lists each sub-tile's
    candidate points (mean ~17, max 32).  Sub-tiles are BIN-PACKED into
    "supergroups" of <= 128 candidate slots (variable per-tile counts -
    no fixed 32-slot padding), 7 supergroups per core, and the 8 cores'
    loads are balanced by splitting a band-major snake of sub-tiles at
    equal-candidate-count boundaries (not fixed 32-row slabs).
  * Per supergroup, TWO matmuls (u-coeffs / v-coeffs stationary [K,128],
    K<=40 shared-row layout: 3 fp16 hi/lo rows per distinct 16-row band,
    3 per distinct 16-col block, 2 const) share ONE moving operand
    [K, 256] (the 16x16 intra-tile pixel offsets) -> one PSUM bank
    [128, 512] = u | v on the SAME partitions.  max(|u|,|v|) therefore
    needs NO partition-realignment copy, and all 128 DVE lanes work.
  * Two chain flavors keep ACT and DVE both busy:
      ACT route: Abs [128,512] (one PSUM pass) -> DVE max fp16.
      DVE route: DMA copies the v half PSUM->SBUF; one DVE
        scalar_tensor_tensor (|u| abs_max |v|) reads u straight from
        PSUM.  (DVE may read one PSUM operand.)
    Then one DVE tensor_scalar min(d,2)-2 -> mr fp16 [128, 256].
  * One canvas matmul per supergroup (K=128 slots, stationary
    [128, 32] = 3g<=30 channel rows zero-padded, moving mr) into a
    2-bank canvas PSUM; sigmoid per bank (scale=4) + output DMA.
  * 3 dummy matmuls on a zeroed region run during the input-DMA wait so
    the PE's HAM activity window starts early (cold 1.2 GHz -> warm
    2.4 GHz after ~3.4us of sustained busy).
"""

import math
import os
import sys

import numpy as np

for _p in ("/opt/trn_rl_repo",):
    if _p not in sys.path and os.path.isdir(_p):
        sys.path.insert(0, _p)

# Geometry (matches the reference module's fixed canvas).
H, W = 256, 384
N_CORES = 8
SUB = 16                                # sub-tile edge (16x16)
N_BANDS = H // SUB                      # 16 row bands
N_CBLK = W // SUB                       # 24 col blocks
F_SUB = SUB * SUB                       # 256 moving columns per group
NSG = 7                                 # supergroups per core
KMAX = 40                               # contraction rows (shared layout)
GMAX = 10                               # sub-tiles per supergroup (3g <= 32)
WIDTH_TO_HEIGHT = 384.0 / 256.0

# Set BASS_KERNEL_TRACE=1 to capture an NTFF profile; results land here.
last_run_info = {}


def _hi_lo(x):
    """Split float64 array into fp16 hi + fp16 lo with tiny residual."""
    hi = x.astype(np.float16)
    lo = (x - hi.astype(np.float64)).astype(np.float16)
    return hi, lo


def _coeffs(locations, matrix_offsets, matrix_scale_exponents):
    loc = np.asarray(locations, np.float64).reshape(-1, 2)      # (N, 2) y,x
    mo = np.asarray(matrix_offsets, np.float64)                  # (N, 2, 2)
    mse = np.asarray(matrix_scale_exponents, np.float64).reshape(-1)
    n = loc.shape[0]

    scale = (math.sqrt(n) / 2.0) / np.exp(mse)
    mats = mo + np.eye(2)[None, :, :] * scale[:, None, None]     # (N, 2, 2)
    b = loc[:, 0, None] * mats[:, 0, :] + loc[:, 1, None] * mats[:, 1, :]

    wy_u = mats[:, 0, 0] + mats[:, 0, 1]
    wx_u = mats[:, 1, 0] + mats[:, 1, 1]
    c_u = -(b[:, 0] + b[:, 1])
    wy_v = mats[:, 0, 0] - mats[:, 0, 1]
    wx_v = mats[:, 1, 0] - mats[:, 1, 1]
    c_v = -(b[:, 0] - b[:, 1])
    return wy_u, wx_u, c_u, wy_v, wx_v, c_v


def _pack(locations, matrix_offsets, matrix_scale_exponents):
    """Cull candidates per 16x16 sub-tile, then bin-pack into supergroups.

    Returns per-core bins: core -> [ (members, K) ] where members is a
    list of (band, cblk, idx_array, slot_offset)."""
    wy_u, wx_u, c_u, wy_v, wx_v, c_v = _coeffs(
        locations, matrix_offsets, matrix_scale_exponents)

    # Preimage bbox of the |u|<=2,|v|<=2 diamond for the y/x SAT axes.
    det = wy_u * wx_v - wx_u * wy_v
    A00 = wx_v / det
    A01 = -wx_u / det
    A10 = -wy_v / det
    A11 = wy_u / det
    y0 = A00 * (-c_u) + A01 * (-c_v)
    x0 = A10 * (-c_u) + A11 * (-c_v)
    hy = 2 * (np.abs(A00) + np.abs(A01))
    hx = 2 * (np.abs(A10) + np.abs(A11))

    ys = np.linspace(-1.0, 1.0, H).astype(np.float32).astype(np.float64)
    xs = np.linspace(-WIDTH_TO_HEIGHT, WIDTH_TO_HEIGHT, W).astype(
        np.float32).astype(np.float64)

    def cull(r0, c0):
        ylo, yhi = ys[r0], ys[r0 + SUB - 1]
        xlo, xhi = xs[c0], xs[c0 + SUB - 1]
        yc, xc = (ylo + yhi) / 2, (xlo + xhi) / 2
        ry, rx = (yhi - ylo) / 2, (xhi - xlo) / 2
        ok = (np.abs(yc - y0) <= ry + hy + 1e-9) & \
             (np.abs(xc - x0) <= rx + hx + 1e-9)
        uc = wy_u * yc + wx_u * xc + c_u
        du = np.abs(wy_u) * ry + np.abs(wx_u) * rx
        ok &= np.abs(uc) <= 2 + du + 1e-9
        vc = wy_v * yc + wx_v * xc + c_v
        dv = np.abs(wy_v) * ry + np.abs(wx_v) * rx
        ok &= np.abs(vc) <= 2 + dv + 1e-9
        return np.nonzero(ok)[0]

    # Band-major snake over sub-tiles, cut into 8 equal-weight runs so the
    # per-core loads balance and each core spans few distinct bands.
    subs = []
    for band in range(N_BANDS):
        rng = range(N_CBLK) if band % 2 == 0 else range(N_CBLK - 1, -1, -1)
        for cb in rng:
            subs.append((band, cb, cull(band * SUB, cb * SUB)))
    msum = sum(len(s[2]) for s in subs)
    target = msum / N_CORES
    runs = [[] for _ in range(N_CORES)]
    acc = 0
    core = 0
    for s in subs:
        if core < N_CORES - 1 and acc + len(s[2]) / 2 > target * (core + 1):
            core += 1
        runs[core].append(s)
        acc += len(s[2])

    def pack_core(items):
        """Worst-fit decreasing into NSG bins under slot/size/K caps."""
        items = sorted(items, key=lambda s: -len(s[2]))
        bins = [{"m": 0, "items": [], "bands": set(), "cols": set()}
                for _ in range(NSG)]
        for s in items:
            m = len(s[2])
            placed = False
            for i in sorted(range(NSG), key=lambda i: bins[i]["m"]):
                bn = bins[i]
                nb = len(bn["bands"] | {s[0]})
                ncb = len(bn["cols"] | {s[1]})
                if (bn["m"] + m <= 128 and len(bn["items"]) < GMAX
                        and 3 * nb + 3 * ncb + 2 <= KMAX):
                    bn["m"] += m
                    bn["items"].append(s)
                    bn["bands"].add(s[0])
                    bn["cols"].add(s[1])
                    placed = True
                    break
            assert placed, "supergroup packing overflow"
        return bins

    cores = []
    for r in runs:
        bins = pack_core(r)
        core_bins = []
        for bn in bins:
            members = []
            off = 0
            for band, cb, idx in bn["items"]:
                members.append((band, cb, idx, off))
                off += len(idx)
            core_bins.append(members)
        cores.append(core_bins)
    return cores


def _prepare(locations, matrix_offsets, matrix_scale_exponents, colors):
    """Host-side prep: coefficients, packing, and device array fill."""
    wy_u, wx_u, c_u, wy_v, wx_v, c_v = _coeffs(
        locations, matrix_offsets, matrix_scale_exponents)
    cols = np.asarray(colors, np.float64).reshape(-1, 3)

    wyu_h, wyu_l = _hi_lo(wy_u)
    wxu_h, wxu_l = _hi_lo(wx_u)
    cu_h, cu_l = _hi_lo(c_u)
    wyv_h, wyv_l = _hi_lo(wy_v)
    wxv_h, wxv_l = _hi_lo(wx_v)
    cv_h, cv_l = _hi_lo(c_v)

    ys = np.linspace(-1.0, 1.0, H).astype(np.float32).astype(np.float64)
    xs = np.linspace(-WIDTH_TO_HEIGHT, WIDTH_TO_HEIGHT, W).astype(
        np.float32).astype(np.float64)
    gyh, gyl = _hi_lo(ys)
    gxh, gxl = _hi_lo(xs)

    pack = _pack(locations, matrix_offsets, matrix_scale_exponents)

    w_np = np.zeros((N_CORES, KMAX, NSG * 256), np.float16)
    g_np = np.zeros((N_CORES, KMAX, NSG * F_SUB), np.float16)
    ct_np = np.zeros((N_CORES, 128, NSG * 32), np.float16)

    for core in range(N_CORES):
        for j, members in enumerate(pack[core]):
            bands = sorted({b for b, _, _, _ in members})
            cblks = sorted({c for _, c, _, _ in members})
            yrow = {b: 3 * i for i, b in enumerate(bands)}
            xbase = 3 * len(bands)
            xrow = {c: xbase + 3 * i for i, c in enumerate(cblks)}
            crow = xbase + 3 * len(cblks)
            assert crow + 2 <= KMAX

            # Moving operand G [KMAX, 256]: per-band y rows (hi, lo, hi),
            # per-colblock x rows (hi, lo, hi), two const rows.
            go = j * F_SUB
            for b in bands:
                r = yrow[b]
                g_np[core, r + 0, go:go + F_SUB] = np.repeat(
                    gyh[b * SUB:(b + 1) * SUB], SUB)
                g_np[core, r + 1, go:go + F_SUB] = np.repeat(
                    gyl[b * SUB:(b + 1) * SUB], SUB)
                g_np[core, r + 2, go:go + F_SUB] = g_np[core, r + 0,
                                                        go:go + F_SUB]
            for c in cblks:
                r = xrow[c]
                g_np[core, r + 0, go:go + F_SUB] = np.tile(
                    gxh[c * SUB:(c + 1) * SUB], SUB)
                g_np[core, r + 1, go:go + F_SUB] = np.tile(
                    gxl[c * SUB:(c + 1) * SUB], SUB)
                g_np[core, r + 2, go:go + F_SUB] = g_np[core, r + 0,
                                                        go:go + F_SUB]
            g_np[core, crow, go:go + F_SUB] = 1.0
            g_np[core, crow + 1, go:go + F_SUB] = 1.0

            # Stationaries: u at cols 256j..+128, v at 256j+128..+128.
            uo = j * 256
            vo = j * 256 + 128
            for ti, (b, c, idx, off) in enumerate(members):
                m = len(idx)
                if m == 0:
                    continue
                sl = slice(off, off + m)
                for o, (wyh_, wyl_, wxh_, wxl_, ch_, cl_) in (
                        (uo, (wyu_h, wyu_l, wxu_h, wxu_l, cu_h, cu_l)),
                        (vo, (wyv_h, wyv_l, wxv_h, wxv_l, cv_h, cv_l))):
                    r = yrow[b]
                    w_np[core, r + 0, o + off:o + off + m] = wyh_[idx]
                    w_np[core, r + 1, o + off:o + off + m] = wyh_[idx]
                    w_np[core, r + 2, o + off:o + off + m] = wyl_[idx]
                    r = xrow[c]
                    w_np[core, r + 0, o + off:o + off + m] = wxh_[idx]
                    w_np[core, r + 1, o + off:o + off + m] = wxh_[idx]
                    w_np[core, r + 2, o + off:o + off + m] = wxl_[idx]
                    w_np[core, crow, o + off:o + off + m] = ch_[idx]
                    w_np[core, crow + 1, o + off:o + off + m] = cl_[idx]
                # Canvas stationary: slot rows off..off+m, channel cols.
                ct_np[core, sl, 32 * j + 3 * ti:32 * j + 3 * ti + 3] = (
                    -0.5 * cols[idx]).astype(np.float16)

    # Sigmoid bias folds the "-2" of mapped'' = min(d,2)-2:
    # sigma(4*(psum - 2*sum(ct))) -> bias[p] = -8*sum_slots ct[:, col(p)].
    b_np = np.zeros((N_CORES, 128, 2), np.float32)
    for core in range(N_CORES):
        colsum = ct_np[core].astype(np.float64).sum(axis=0)  # [NSG*32]
        for j in range(NSG):
            bank, s = (0, j) if j < 4 else (1, j - 4)
            b_np[core, 32 * s:32 * s + 32, bank] = (
                -8.0 * colsum[32 * j:32 * j + 32])

    return w_np, g_np, ct_np, b_np, pack


def _build_nc():
    """Build the Bass/Tile program (shared by all cores)."""
    from contextlib import ExitStack

    import concourse.bacc as bacc
    import concourse.tile as tile
    from concourse import mybir

    f16 = mybir.dt.float16
    f32 = mybir.dt.float32
    nc = bacc.Bacc("TRN2", target_bir_lowering=False, debug=False,
                   num_devices=N_CORES)

    w_d = nc.dram_tensor("w", [KMAX, NSG * 256], f16, kind="ExternalInput")
    g_d = nc.dram_tensor("g", [KMAX, NSG * F_SUB], f16, kind="ExternalInput")
    ct_d = nc.dram_tensor("ct", [128, NSG * 32], f16, kind="ExternalInput")
    b_d = nc.dram_tensor("b", [128, 2], f32, kind="ExternalInput")
    # y[bank, slot_partition, px]
    y_d = nc.dram_tensor("y", [2, 128, F_SUB], f16, kind="ExternalOutput")

    ROUTE_D = {2, 3, 6}      # all-DVE route (reads PSUM at fp32 rate);
    # {0,1} get solo ACT Abs passes, {4,5} a pair-batched one.
    # Processing order: sg 6 runs BEFORE 4,5 so its chain (and bank 1's
    # last canvas matmul) isn't serialized behind the pair-batched Abs.
    ORDER = [0, 1, 2, 3, 6, 4, 5]
    DELAY = 4                # canvas matmuls trail the uv stream

    with ExitStack() as ctx:
        tc = ctx.enter_context(tile.TileContext(nc))
        const = ctx.enter_context(tc.tile_pool(name="const", bufs=1))
        uvpool = ctx.enter_context(tc.tile_pool(name="uv", bufs=1,
                                                space="PSUM"))
        cvpool = ctx.enter_context(tc.tile_pool(name="cv", bufs=1,
                                                space="PSUM"))
        abpool = ctx.enter_context(tc.tile_pool(name="ab", bufs=2))
        vpool = ctx.enter_context(tc.tile_pool(name="vs", bufs=3))
        dpool = ctx.enter_context(tc.tile_pool(name="dm", bufs=3))
        rpool = ctx.enter_context(tc.tile_pool(name="mr", bufs=6))
        opool = ctx.enter_context(tc.tile_pool(name="o", bufs=1))

        W_sb = const.tile([KMAX, NSG * 256], f16)
        G_sb = const.tile([KMAX, NSG * F_SUB], f16)
        CT_sb = const.tile([128, NSG * 32], f16)
        B_sb = const.tile([128, 2], f32)
        # Tiny zeroed region feeding the act-table-pinning sigmoid.
        # (PE warmup matmuls were tried and dropped: on this part the HAM
        # clock-gate never releases - matmuls run at 1.2 GHz throughout -
        # so warmups only delayed the real stream.)
        warm = const.tile([128, 16], f16)

        # Input DMA split: the scalar queue boots straight into a DMA (no
        # preamble memsets like gpsimd), so the g-lead lands earliest
        # there - it would otherwise gate the first matmul; the act-table
        # load queues right behind it, still well before the first Abs.
        # The memset for the table-pinning sigmoid goes on the (otherwise
        # idle until mid-kernel) vector engine.
        nc.vector.memset(warm[:], 0.0)
        nc.scalar.dma_start(G_sb[:, 0:512], g_d[:, 0:512])
        nc.sync.dma_start(W_sb[:, 0:512], w_d[:, 0:512])
        nc.gpsimd.dma_start(G_sb[:, 512:1152], g_d[:, 512:1152])
        nc.sync.dma_start(W_sb[:, 512:1792], w_d[:, 512:1792])
        nc.gpsimd.dma_start(G_sb[:, 1152:1792], g_d[:, 1152:1792])
        nc.gpsimd.dma_start(CT_sb[:], ct_d[:])
        nc.gpsimd.dma_start(B_sb[:], b_d[:])

        # Canvas PSUM: bank 0 = supergroups 0-3, bank 1 = 4-6.
        cv0 = cvpool.tile([128, 512], f32, tag="c0", bufs=1)
        cv1 = cvpool.tile([128, 512], f32, tag="c1", bufs=1)
        outr0 = opool.tile([128, F_SUB], f16, tag="o0", bufs=1)
        outr1 = opool.tile([128, F_SUB], f16, tag="o1", bufs=1)

        # Pin the act-table set that holds BOTH Abs and Sigmoid by issuing
        # a tiny sigmoid first; Abs and the final sigmoids then share one
        # table and no mid-kernel ACT_TABLE_LOAD lands on the chain.
        warmo = opool.tile([128, 1], f32, tag="wo", bufs=1)
        nc.scalar.activation(warmo[:], warm[:, 0:1],
                             mybir.ActivationFunctionType.Sigmoid)

        # u|v PSUM layout (8 banks exactly, with cv0/cv1):
        #  - sgs 0 and 1: own 1-bank tiles with SOLO Abs passes.  Early in
        #    the kernel ACT has slack, and a solo Abs right after sg0's
        #    matmuls starts the DVE chain ~0.7us earlier than a pair-
        #    batched one (which must wait for sg1's matmuls too).
        #  - sgs 4,5: one 2-bank tile, pair-batched Abs (mid-kernel ACT is
        #    the busy engine, so batching saves its per-op overhead).
        #  - sgs {2,3} (DVE reduce route) and 6 (solo Abs): a rotating
        #    2x1-bank pool.
        p0 = uvpool.tile([128, 512], f32, tag="p0", bufs=1)
        p1 = uvpool.tile([128, 512], f32, tag="p1", bufs=1)
        pairB = uvpool.tile([128, 1024], f32, tag="pb", bufs=1)
        pair_of = {0: (p0, 0), 1: (p1, 0), 4: (pairB, 0), 5: (pairB, 512)}
        # sg 6 reuses sg 0's bank (free once sg 0's Abs has read it).
        dtile = {NSG - 1: p0}
        for j in (2, 3):
            dtile[j] = uvpool.tile([128, 512], f32, tag="pd", bufs=2,
                                   name=f"pd{j}")

        bank_left = [4, 3]

        def canvas_mm(j, mr):
            if j < 4:
                cv, s = cv0, j
            else:
                cv, s = cv1, j - 4
            nc.tensor.matmul(cv[32 * s:32 * s + 32, 0:F_SUB],
                             CT_sb[:, 32 * j:32 * j + 32], mr,
                             start=True, stop=True,
                             tile_position=(0, 32 * s))
            # canvas holds sum(ct * min(d,2)); the "-2" term is folded into
            # the per-partition sigmoid bias (bias = -8*sum(ct)).
            bank = 0 if j < 4 else 1
            bank_left[bank] -= 1
            if bank_left[bank]:
                return
            if bank == 0:
                nc.scalar.activation(outr0[:], cv0[:, 0:F_SUB],
                                     mybir.ActivationFunctionType.Sigmoid,
                                     bias=B_sb[:, 0:1], scale=4.0)
                nc.sync.dma_start(y_d[0], outr0[:])
            else:
                nc.scalar.activation(outr1[0:96, :], cv1[0:96, 0:F_SUB],
                                     mybir.ActivationFunctionType.Sigmoid,
                                     bias=B_sb[0:96, 1:2], scale=4.0)
                # Final output: split across the sync and scalar queues -
                # this DMA is pure tail latency (scalar is free after the
                # last sigmoid; gpsimd sees the sigmoid's semaphore ~0.4us
                # later than the others, so it gets no slice).
                nc.sync.dma_start(y_d[1, 0:48], outr1[0:48, :])
                nc.scalar.dma_start(y_d[1, 48:96], outr1[48:96, :])

        def clamp(dm):
            """min(d,2). (gpsimd was tried for this and is ~20x slower -
            its tensor ops are software DSP loops - so it stays on DVE.)"""
            mr = rpool.tile([128, 256], f16, tag="mr")
            nc.vector.tensor_scalar(mr[:], dm[:], 2.0, None,
                                    op0=mybir.AluOpType.min)
            return mr

        def max_min(ab, o):
            """fp16 max over the u|v column halves, then clamp at 2."""
            dm = dpool.tile([128, 256], f16, tag="dm")
            nc.vector.tensor_tensor(dm[:], ab[:, o:o + 256],
                                    ab[:, o + 256:o + 512],
                                    op=mybir.AluOpType.max)
            return clamp(dm)[:]

        def dve_route(pt):
            """max(|u|,|v|) straight from PSUM: one reduce over the uv axis."""
            view = pt[:, 0:512].rearrange("p (uv px) -> p px uv", uv=2)
            dm = dpool.tile([128, 256], f16, tag="dm")
            nc.vector.tensor_reduce(dm[:], view, axis=mybir.AxisListType.X,
                                    op=mybir.AluOpType.max,
                                    apply_absolute_value=True)
            return clamp(dm)[:]

        pend = []

        def flush(limit):
            while len(pend) > limit:
                canvas_mm(*pend.pop(0))

        for j in ORDER:
            if j in dtile:
                pt, cs = dtile[j], 0
            else:
                pt, cs = pair_of[j]
            nc.tensor.matmul(pt[:, cs:cs + 256],
                             W_sb[:, 256 * j:256 * j + 128],
                             G_sb[:, F_SUB * j:F_SUB * (j + 1)],
                             start=True, stop=True)
            nc.tensor.matmul(pt[:, cs + 256:cs + 512],
                             W_sb[:, 256 * j + 128:256 * (j + 1)],
                             G_sb[:, F_SUB * j:F_SUB * (j + 1)],
                             start=True, stop=True)
            if j in (0, 1):
                # Solo Abs chain.
                ab = abpool.tile([128, 512], f16, tag="ab")
                nc.scalar.activation(ab[:], pt[:],
                                     mybir.ActivationFunctionType.Abs)
                pend.append((j, max_min(ab, 0)))
            elif j == 5:
                # Pair-batched Abs over sgs 4 and 5, plus one batched
                # max-pair clamp: both dm halves land in one tile so a
                # single [128,512] min op covers both sgs.
                ab = abpool.tile([128, 1024], f16, tag="ab2")
                nc.scalar.activation(ab[:], pt[:],
                                     mybir.ActivationFunctionType.Abs)
                dm2 = dpool.tile([128, 512], f16, tag="dm2")
                nc.vector.tensor_tensor(dm2[:, 0:256], ab[:, 0:256],
                                        ab[:, 256:512],
                                        op=mybir.AluOpType.max)
                nc.vector.tensor_tensor(dm2[:, 256:512], ab[:, 512:768],
                                        ab[:, 768:1024],
                                        op=mybir.AluOpType.max)
                mr2 = rpool.tile([128, 512], f16, tag="mr2")
                nc.vector.tensor_scalar(mr2[:], dm2[:], 2.0, None,
                                        op0=mybir.AluOpType.min)
                pend.append((4, mr2[:, 0:256]))
                pend.append((5, mr2[:, 256:512]))
            elif j in ROUTE_D:
                pend.append((j, dve_route(pt)))
            flush(DELAY)
        flush(0)

    nc.compile()
    return nc


def _install_ntff_hook():
    """Provide antenv.axon_hooks if the image lacks it (ctypes shim around
    libaxon_pjrt.so's NRT profile capture). Returns True on success."""
    try:
        from antenv.axon_hooks import get_axon_ntff_profile_hook  # noqa: F401
        return True
    except ImportError:
        pass
    try:
        import contextlib
        import ctypes
        import types

        import antenv

        so_path = "/opt/axon/libaxon_pjrt.so"
        lib = ctypes.CDLL(so_path)
        if not hasattr(lib, "axon_start_nrt_profile"):
            return False
        lib.axon_start_nrt_profile.argtypes = [
            ctypes.POINTER(ctypes.c_int64), ctypes.c_size_t]
        lib.axon_start_nrt_profile.restype = ctypes.c_int64
        lib.axon_stop_nrt_profile.argtypes = [ctypes.c_char_p]
        lib.axon_stop_nrt_profile.restype = ctypes.c_int64

        @contextlib.contextmanager
        def _hook(output_dir, device_ids):
            import jax
            jax.devices()
            if device_ids:
                ids = (ctypes.c_int64 * len(device_ids))(*device_ids)
                rc = lib.axon_start_nrt_profile(ids, len(device_ids))
            else:
                rc = lib.axon_start_nrt_profile(None, 0)
            if rc != 0:
                raise RuntimeError(f"axon_start_nrt_profile rc={rc}")
            try:
                yield
            finally:
                n = lib.axon_stop_nrt_profile(str(output_dir).encode())
                print(f"ntff profile: {n} file(s) -> {output_dir}", file=sys.stderr)

        mod = types.ModuleType("antenv.axon_hooks")
        mod._hook = _hook
        mod.get_axon_ntff_profile_hook = lambda: _hook
        mod.set_axon_ntff_profile_hook = lambda h: None
        sys.modules["antenv.axon_hooks"] = mod
        antenv.axon_hooks = mod
        return True
    except Exception as e:  # pragma: no cover
        print("ntff hook install failed:", e, file=sys.stderr)
        return False


def _unshard(results, pack):
    """Reassemble per-core y [2, 128, 256] into the full (3, H, W)."""
    out = np.empty((3, H, W), np.float32)
    for core in range(N_CORES):
        y = np.asarray(results[core]["y"], np.float32)   # [2, 128, 256]
        for j, members in enumerate(pack[core]):
            bank, s = (0, j) if j < 4 else (1, j - 4)
            for ti, (band, cb, idx, off) in enumerate(members):
                blk = y[bank, 32 * s + 3 * ti:32 * s + 3 * ti + 3, :]
                out[:, band * SUB:(band + 1) * SUB,
                    cb * SUB:(cb + 1) * SUB] = blk.reshape(3, SUB, SUB)
    return out


def kernel(locations, matrix_offsets, matrix_scale_exponents, colors,
           canvas_height_px, canvas_width_px):
    assert int(canvas_height_px) == H and int(canvas_width_px) == W

    w_np, g_np, ct_np, b_np, pack = _prepare(
        locations, matrix_offsets, matrix_scale_exponents, colors)

    nc = _build_nc()

    from concourse.bass_utils import run_bass_kernel_spmd

    in_maps = [
        {"w": w_np[c], "g": g_np[c], "ct": ct_np[c], "b": b_np[c]}
        for c in range(N_CORES)
    ]
    trace = bool(int(os.environ.get("BASS_KERNEL_TRACE", "1")))
    if trace:
        trace = _install_ntff_hook()
    try:
        res = run_bass_kernel_spmd(nc, in_maps, core_ids=list(range(N_CORES)),
                                   trace=trace)
    except Exception:
        if not trace:
            raise
        res = run_bass_kernel_spmd(nc, in_maps, core_ids=list(range(N_CORES)),
                                   trace=False)
    last_run_info.clear()
    last_run_info.update(
        exec_time_ns=res.exec_time_ns,
        mean_exec_time_ns=res.mean_exec_time_ns,
        profile_json=res.profile_json,
    )

    return _unshard(res.results, pack)


# revision 47
# speedup vs baseline: 1.0074x; 1.0074x over previous
"""Trainium2 Bass kernel for the nn_Points problem.

Renders N=1024 anisotropic "diamond" points onto a 3x256x384 canvas:
    t = (pixel - loc) @ M_n          (2-vector per pixel per point)
    mapped = relu(1 - (|t0|+|t1|)/2)
    canvas = sigmoid(4 * sum_n mapped * color_n)

Design (8 NeuronCores, full inputs in / full output out):
  * u = t0+t1, v = t0-t1, so |t0|+|t1| = max(|u|,|v|) and
    mapped'' := min(max(|u|,|v|), 2) - 2 = -2*mapped (colors pre-scaled
    by -c/2 make the canvas matmul come out right).
  * The image is cut into 384 16x16 sub-tiles; an exact SAT cull (tile
    rect vs the preimage of the |t|_1<=2 diamond) lists each sub-tile's
    candidate points (mean ~17, max 32).  Sub-tiles are BIN-PACKED into
    "supergroups" of <= 128 candidate slots (variable per-tile counts -
    no fixed 32-slot padding), 7 supergroups per core, and the 8 cores'
    loads are balanced by splitting a band-major snake of sub-tiles at
    equal-candidate-count boundaries (not fixed 32-row slabs).
  * Per supergroup, TWO matmuls (u-coeffs / v-coeffs stationary [K,128],
    K<=40 shared-row layout: 3 fp16 hi/lo rows per distinct 16-row band,
    3 per distinct 16-col block, 2 const) share ONE moving operand
    [K, 256] (the 16x16 intra-tile pixel offsets) -> one PSUM bank
    [128, 512] = u | v on the SAME partitions.  max(|u|,|v|) therefore
    needs NO partition-realignment copy, and all 128 DVE lanes work.
  * Two chain flavors keep ACT and DVE both busy:
      ACT route: Abs [128,512] (one PSUM pass) -> DVE max fp16.
      DVE route: DMA copies the v half PSUM->SBUF; one DVE
        scalar_tensor_tensor (|u| abs_max |v|) reads u straight from
        PSUM.  (DVE may read one PSUM operand.)
    Then one DVE tensor_scalar min(d,2)-2 -> mr fp16 [128, 256].
  * One canvas matmul per supergroup (K=128 slots, stationary
    [128, 32] = 3g<=30 channel rows zero-padded, moving mr) into a
    2-bank canvas PSUM; sigmoid per bank (scale=4) + output DMA.
  * 3 dummy matmuls on a zeroed region run during the input-DMA wait so
    the PE's HAM activity window starts early (cold 1.2 GHz -> warm
    2.4 GHz after ~3.4us of sustained busy).
"""

import math
import os
import sys

import numpy as np

for _p in ("/opt/trn_rl_repo",):
    if _p not in sys.path and os.path.isdir(_p):
        sys.path.insert(0, _p)

# Geometry (matches the reference module's fixed canvas).
H, W = 256, 384
N_CORES = 8
SUB = 16                                # sub-tile edge (16x16)
N_BANDS = H // SUB                      # 16 row bands
N_CBLK = W // SUB                       # 24 col blocks
F_SUB = SUB * SUB                       # 256 moving columns per group
NSG = 7                                 # supergroups per core
KMAX = 40                               # contraction rows (shared layout)
GMAX = 10                               # sub-tiles per supergroup (3g <= 32)
WIDTH_TO_HEIGHT = 384.0 / 256.0

# Set BASS_KERNEL_TRACE=1 to capture an NTFF profile; results land here.
last_run_info = {}


def _hi_lo(x):
    """Split float64 array into fp16 hi + fp16 lo with tiny residual."""
    hi = x.astype(np.float16)
    lo = (x - hi.astype(np.float64)).astype(np.float16)
    return hi, lo


def _coeffs(locations, matrix_offsets, matrix_scale_exponents):
    loc = np.asarray(locations, np.float64).reshape(-1, 2)      # (N, 2) y,x
    mo = np.asarray(matrix_offsets, np.float64)                  # (N, 2, 2)
    mse = np.asarray(matrix_scale_exponents, np.float64).reshape(-1)
    n = loc.shape[0]

    scale = (math.sqrt(n) / 2.0) / np.exp(mse)
    mats = mo + np.eye(2)[None, :, :] * scale[:, None, None]     # (N, 2, 2)
    b = loc[:, 0, None] * mats[:, 0, :] + loc[:, 1, None] * mats[:, 1, :]

    wy_u = mats[:, 0, 0] + mats[:, 0, 1]
    wx_u = mats[:, 1, 0] + mats[:, 1, 1]
    c_u = -(b[:, 0] + b[:, 1])
    wy_v = mats[:, 0, 0] - mats[:, 0, 1]
    wx_v = mats[:, 1, 0] - mats[:, 1, 1]
    c_v = -(b[:, 0] - b[:, 1])
    return wy_u, wx_u, c_u, wy_v, wx_v, c_v


def _pack(locations, matrix_offsets, matrix_scale_exponents):
    """Cull candidates per 16x16 sub-tile, then bin-pack into supergroups.

    Returns per-core bins: core -> [ (members, K) ] where members is a
    list of (band, cblk, idx_array, slot_offset)."""
    wy_u, wx_u, c_u, wy_v, wx_v, c_v = _coeffs(
        locations, matrix_offsets, matrix_scale_exponents)

    # Preimage bbox of the |u|<=2,|v|<=2 diamond for the y/x SAT axes.
    det = wy_u * wx_v - wx_u * wy_v
    A00 = wx_v / det
    A01 = -wx_u / det
    A10 = -wy_v / det
    A11 = wy_u / det
    y0 = A00 * (-c_u) + A01 * (-c_v)
    x0 = A10 * (-c_u) + A11 * (-c_v)
    hy = 2 * (np.abs(A00) + np.abs(A01))
    hx = 2 * (np.abs(A10) + np.abs(A11))

    ys = np.linspace(-1.0, 1.0, H).astype(np.float32).astype(np.float64)
    xs = np.linspace(-WIDTH_TO_HEIGHT, WIDTH_TO_HEIGHT, W).astype(
        np.float32).astype(np.float64)

    def cull(r0, c0):
        ylo, yhi = ys[r0], ys[r0 + SUB - 1]
        xlo, xhi = xs[c0], xs[c0 + SUB - 1]
        yc, xc = (ylo + yhi) / 2, (xlo + xhi) / 2
        ry, rx = (yhi - ylo) / 2, (xhi - xlo) / 2
        ok = (np.abs(yc - y0) <= ry + hy + 1e-9) & \
             (np.abs(xc - x0) <= rx + hx + 1e-9)
        uc = wy_u * yc + wx_u * xc + c_u
        du = np.abs(wy_u) * ry + np.abs(wx_u) * rx
        ok &= np.abs(uc) <= 2 + du + 1e-9
        vc = wy_v * yc + wx_v * xc + c_v
        dv = np.abs(wy_v) * ry + np.abs(wx_v) * rx
        ok &= np.abs(vc) <= 2 + dv + 1e-9
        return np.nonzero(ok)[0]

    # Band-major snake over sub-tiles, cut into 8 equal-weight runs so the
    # per-core loads balance and each core spans few distinct bands.
    subs = []
    for band in range(N_BANDS):
        rng = range(N_CBLK) if band % 2 == 0 else range(N_CBLK - 1, -1, -1)
        for cb in rng:
            subs.append((band, cb, cull(band * SUB, cb * SUB)))
    msum = sum(len(s[2]) for s in subs)
    target = msum / N_CORES
    runs = [[] for _ in range(N_CORES)]
    acc = 0
    core = 0
    for s in subs:
        if core < N_CORES - 1 and acc + len(s[2]) / 2 > target * (core + 1):
            core += 1
        runs[core].append(s)
        acc += len(s[2])

    def pack_core(items):
        """Worst-fit decreasing into NSG bins under slot/size/K caps."""
        items = sorted(items, key=lambda s: -len(s[2]))
        bins = [{"m": 0, "items": [], "bands": set(), "cols": set()}
                for _ in range(NSG)]
        for s in items:
            m = len(s[2])
            placed = False
            for i in sorted(range(NSG), key=lambda i: bins[i]["m"]):
                bn = bins[i]
                nb = len(bn["bands"] | {s[0]})
                ncb = len(bn["cols"] | {s[1]})
                if (bn["m"] + m <= 128 and len(bn["items"]) < GMAX
                        and 3 * nb + 3 * ncb + 2 <= KMAX):
                    bn["m"] += m
                    bn["items"].append(s)
                    bn["bands"].add(s[0])
                    bn["cols"].add(s[1])
                    placed = True
                    break
            assert placed, "supergroup packing overflow"
        return bins

    cores = []
    for r in runs:
        bins = pack_core(r)
        core_bins = []
        for bn in bins:
            members = []
            off = 0
            for band, cb, idx in bn["items"]:
                members.append((band, cb, idx, off))
                off += len(idx)
            core_bins.append(members)
        cores.append(core_bins)
    return cores


def _prepare(locations, matrix_offsets, matrix_scale_exponents, colors):
    """Host-side prep: coefficients, packing, and device array fill."""
    wy_u, wx_u, c_u, wy_v, wx_v, c_v = _coeffs(
        locations, matrix_offsets, matrix_scale_exponents)
    cols = np.asarray(colors, np.float64).reshape(-1, 3)

    wyu_h, wyu_l = _hi_lo(wy_u)
    wxu_h, wxu_l = _hi_lo(wx_u)
    cu_h, cu_l = _hi_lo(c_u)
    wyv_h, wyv_l = _hi_lo(wy_v)
    wxv_h, wxv_l = _hi_lo(wx_v)
    cv_h, cv_l = _hi_lo(c_v)

    ys = np.linspace(-1.0, 1.0, H).astype(np.float32).astype(np.float64)
    xs = np.linspace(-WIDTH_TO_HEIGHT, WIDTH_TO_HEIGHT, W).astype(
        np.float32).astype(np.float64)
    gyh, gyl = _hi_lo(ys)
    gxh, gxl = _hi_lo(xs)

    pack = _pack(locations, matrix_offsets, matrix_scale_exponents)

    w_np = np.zeros((N_CORES, KMAX, NSG * 256), np.float16)
    g_np = np.zeros((N_CORES, KMAX, NSG * F_SUB), np.float16)
    ct_np = np.zeros((N_CORES, 128, NSG * 32), np.float16)

    for core in range(N_CORES):
        for j, members in enumerate(pack[core]):
            bands = sorted({b for b, _, _, _ in members})
            cblks = sorted({c for _, c, _, _ in members})
            yrow = {b: 3 * i for i, b in enumerate(bands)}
            xbase = 3 * len(bands)
            xrow = {c: xbase + 3 * i for i, c in enumerate(cblks)}
            crow = xbase + 3 * len(cblks)
            assert crow + 2 <= KMAX

            # Moving operand G [KMAX, 256]: per-band y rows (hi, lo, hi),
            # per-colblock x rows (hi, lo, hi), two const rows.
            go = j * F_SUB
            for b in bands:
                r = yrow[b]
                g_np[core, r + 0, go:go + F_SUB] = np.repeat(
                    gyh[b * SUB:(b + 1) * SUB], SUB)
                g_np[core, r + 1, go:go + F_SUB] = np.repeat(
                    gyl[b * SUB:(b + 1) * SUB], SUB)
                g_np[core, r + 2, go:go + F_SUB] = g_np[core, r + 0,
                                                        go:go + F_SUB]
            for c in cblks:
                r = xrow[c]
                g_np[core, r + 0, go:go + F_SUB] = np.tile(
                    gxh[c * SUB:(c + 1) * SUB], SUB)
                g_np[core, r + 1, go:go + F_SUB] = np.tile(
                    gxl[c * SUB:(c + 1) * SUB], SUB)
                g_np[core, r + 2, go:go + F_SUB] = g_np[core, r + 0,
                                                        go:go + F_SUB]
            g_np[core, crow, go:go + F_SUB] = 1.0
            g_np[core, crow + 1, go:go + F_SUB] = 1.0

            # Stationaries: u at cols 256j..+128, v at 256j+128..+128.
            uo = j * 256
            vo = j * 256 + 128
            for ti, (b, c, idx, off) in enumerate(members):
                m = len(idx)
                if m == 0:
                    continue
                sl = slice(off, off + m)
                for o, (wyh_, wyl_, wxh_, wxl_, ch_, cl_) in (
                        (uo, (wyu_h, wyu_l, wxu_h, wxu_l, cu_h, cu_l)),
                        (vo, (wyv_h, wyv_l, wxv_h, wxv_l, cv_h, cv_l))):
                    r = yrow[b]
                    w_np[core, r + 0, o + off:o + off + m] = wyh_[idx]
                    w_np[core, r + 1, o + off:o + off + m] = wyh_[idx]
                    w_np[core, r + 2, o + off:o + off + m] = wyl_[idx]
                    r = xrow[c]
                    w_np[core, r + 0, o + off:o + off + m] = wxh_[idx]
                    w_np[core, r + 1, o + off:o + off + m] = wxh_[idx]
                    w_np[core, r + 2, o + off:o + off + m] = wxl_[idx]
                    w_np[core, crow, o + off:o + off + m] = ch_[idx]
                    w_np[core, crow + 1, o + off:o + off + m] = cl_[idx]
                # Canvas stationary: slot rows off..off+m, channel cols.
                ct_np[core, sl, 32 * j + 3 * ti:32 * j + 3 * ti + 3] = (
                    -0.5 * cols[idx]).astype(np.float16)

    # Sigmoid bias folds the "-2" of mapped'' = min(d,2)-2:
    # sigma(4*(psum - 2*sum(ct))) -> bias[p] = -8*sum_slots ct[:, col(p)].
    b_np = np.zeros((N_CORES, 128, 2), np.float32)
    for core in range(N_CORES):
        colsum = ct_np[core].astype(np.float64).sum(axis=0)  # [NSG*32]
        for j in range(NSG):
            bank, s = (0, j) if j < 4 else (1, j - 4)
            b_np[core, 32 * s:32 * s + 32, bank] = (
                -8.0 * colsum[32 * j:32 * j + 32])

    return w_np, g_np, ct_np, b_np, pack


def _build_nc():
    """Build the Bass/Tile program (shared by all cores)."""
    from contextlib import ExitStack

    import concourse.bacc as bacc
    import concourse.tile as tile
    from concourse import mybir

    f16 = mybir.dt.float16
    f32 = mybir.dt.float32
    nc = bacc.Bacc("TRN2", target_bir_lowering=False, debug=False,
                   num_devices=N_CORES)

    w_d = nc.dram_tensor("w", [KMAX, NSG * 256], f16, kind="ExternalInput")
    g_d = nc.dram_tensor("g", [KMAX, NSG * F_SUB], f16, kind="ExternalInput")
    ct_d = nc.dram_tensor("ct", [128, NSG * 32], f16, kind="ExternalInput")
    b_d = nc.dram_tensor("b", [128, 2], f32, kind="ExternalInput")
    # y[bank, slot_partition, px]
    y_d = nc.dram_tensor("y", [2, 128, F_SUB], f16, kind="ExternalOutput")

    ROUTE_D = {2, 3}         # all-DVE route (reads PSUM at fp32 rate);
    # {0,1} get solo ACT Abs passes, {4,5} a pair-batched one, {6} a solo
    # Abs (late in the kernel ACT frees up first while DVE has backlog).
    ORDER = [0, 1, 2, 3, 4, 5, 6]
    DELAY = 4                # canvas matmuls trail the uv stream

    with ExitStack() as ctx:
        tc = ctx.enter_context(tile.TileContext(nc))
        const = ctx.enter_context(tc.tile_pool(name="const", bufs=1))
        uvpool = ctx.enter_context(tc.tile_pool(name="uv", bufs=1,
                                                space="PSUM"))
        cvpool = ctx.enter_context(tc.tile_pool(name="cv", bufs=1,
                                                space="PSUM"))
        abpool = ctx.enter_context(tc.tile_pool(name="ab", bufs=2))
        vpool = ctx.enter_context(tc.tile_pool(name="vs", bufs=3))
        dpool = ctx.enter_context(tc.tile_pool(name="dm", bufs=3))
        rpool = ctx.enter_context(tc.tile_pool(name="mr", bufs=6))
        opool = ctx.enter_context(tc.tile_pool(name="o", bufs=1))

        W_sb = const.tile([KMAX, NSG * 256], f16)
        G_sb = const.tile([KMAX, NSG * F_SUB], f16)
        CT_sb = const.tile([128, NSG * 32], f16)
        B_sb = const.tile([128, 2], f32)
        # Tiny zeroed region feeding the act-table-pinning sigmoid.
        # (PE warmup matmuls were tried and dropped: on this part the HAM
        # clock-gate never releases - matmuls run at 1.2 GHz throughout -
        # so warmups only delayed the real stream.)
        warm = const.tile([128, 16], f16)

        # Input DMA split: the scalar queue boots straight into a DMA (no
        # preamble memsets like gpsimd), so the g-lead lands earliest
        # there - it would otherwise gate the first matmul; the act-table
        # load queues right behind it, still well before the first Abs.
        # The memset for the table-pinning sigmoid goes on the (otherwise
        # idle until mid-kernel) vector engine.
        nc.vector.memset(warm[:], 0.0)
        nc.scalar.dma_start(G_sb[:, 0:512], g_d[:, 0:512])
        nc.sync.dma_start(W_sb[:, 0:512], w_d[:, 0:512])
        nc.gpsimd.dma_start(G_sb[:, 512:1152], g_d[:, 512:1152])
        nc.sync.dma_start(W_sb[:, 512:1792], w_d[:, 512:1792])
        nc.gpsimd.dma_start(G_sb[:, 1152:1792], g_d[:, 1152:1792])
        nc.gpsimd.dma_start(CT_sb[:], ct_d[:])
        nc.gpsimd.dma_start(B_sb[:], b_d[:])

        # Canvas PSUM: bank 0 = supergroups 0-3, bank 1 = 4-6.
        cv0 = cvpool.tile([128, 512], f32, tag="c0", bufs=1)
        cv1 = cvpool.tile([128, 512], f32, tag="c1", bufs=1)
        outr0 = opool.tile([128, F_SUB], f16, tag="o0", bufs=1)
        outr1 = opool.tile([128, F_SUB], f16, tag="o1", bufs=1)

        # Pin the act-table set that holds BOTH Abs and Sigmoid by issuing
        # a tiny sigmoid first; Abs and the final sigmoids then share one
        # table and no mid-kernel ACT_TABLE_LOAD lands on the chain.
        warmo = opool.tile([128, 1], f32, tag="wo", bufs=1)
        nc.scalar.activation(warmo[:], warm[:, 0:1],
                             mybir.ActivationFunctionType.Sigmoid)

        # u|v PSUM layout (8 banks exactly, with cv0/cv1):
        #  - sgs 0 and 1: own 1-bank tiles with SOLO Abs passes.  Early in
        #    the kernel ACT has slack, and a solo Abs right after sg0's
        #    matmuls starts the DVE chain ~0.7us earlier than a pair-
        #    batched one (which must wait for sg1's matmuls too).
        #  - sgs 4,5: one 2-bank tile, pair-batched Abs (mid-kernel ACT is
        #    the busy engine, so batching saves its per-op overhead).
        #  - sgs {2,3} (DVE reduce route) and 6 (solo Abs): a rotating
        #    2x1-bank pool.
        p0 = uvpool.tile([128, 512], f32, tag="p0", bufs=1)
        p1 = uvpool.tile([128, 512], f32, tag="p1", bufs=1)
        pairB = uvpool.tile([128, 1024], f32, tag="pb", bufs=1)
        pair_of = {0: (p0, 0), 1: (p1, 0), 4: (pairB, 0), 5: (pairB, 512)}
        # sg 6 (solo Abs) reuses sg 0's bank - free once sg 0's Abs read it.
        dtile = {NSG - 1: p0}
        for j in (2, 3):
            dtile[j] = uvpool.tile([128, 512], f32, tag="pd", bufs=2,
                                   name=f"pd{j}")

        bank_left = [4, 3]

        def canvas_mm(j, mr):
            if j < 4:
                cv, s = cv0, j
            else:
                cv, s = cv1, j - 4
            # Deprioritized so the tile scheduler never lets a canvas
            # matmul preempt the uv stream (the PE is free afterwards,
            # while a delayed uv matmul delays its whole chain).
            with tc.high_priority(offset=-100000):
                nc.tensor.matmul(cv[32 * s:32 * s + 32, 0:F_SUB],
                                 CT_sb[:, 32 * j:32 * j + 32], mr,
                                 start=True, stop=True,
                                 tile_position=(0, 32 * s))
            # canvas holds sum(ct * min(d,2)); the "-2" term is folded into
            # the per-partition sigmoid bias (bias = -8*sum(ct)).
            bank = 0 if j < 4 else 1
            bank_left[bank] -= 1
            if bank_left[bank]:
                return
            if bank == 0:
                nc.scalar.activation(outr0[:], cv0[:, 0:F_SUB],
                                     mybir.ActivationFunctionType.Sigmoid,
                                     bias=B_sb[:, 0:1], scale=4.0)
                nc.sync.dma_start(y_d[0], outr0[:])
            else:
                nc.scalar.activation(outr1[0:96, :], cv1[0:96, 0:F_SUB],
                                     mybir.ActivationFunctionType.Sigmoid,
                                     bias=B_sb[0:96, 1:2], scale=4.0)
                # Final output: split across the sync and scalar queues -
                # this DMA is pure tail latency (scalar is free after the
                # last sigmoid; gpsimd sees the sigmoid's semaphore ~0.4us
                # later than the others, so it gets no slice).
                nc.sync.dma_start(y_d[1, 0:48], outr1[0:48, :])
                nc.scalar.dma_start(y_d[1, 48:96], outr1[48:96, :])

        def clamp(dm):
            """min(d,2). (gpsimd was tried for this and is ~20x slower -
            its tensor ops are software DSP loops - so it stays on DVE.)"""
            mr = rpool.tile([128, 256], f16, tag="mr")
            nc.vector.tensor_scalar(mr[:], dm[:], 2.0, None,
                                    op0=mybir.AluOpType.min)
            return mr

        def max_min(ab, o):
            """fp16 max over the u|v column halves, then clamp at 2."""
            dm = dpool.tile([128, 256], f16, tag="dm")
            nc.vector.tensor_tensor(dm[:], ab[:, o:o + 256],
                                    ab[:, o + 256:o + 512],
                                    op=mybir.AluOpType.max)
            return clamp(dm)[:]

        def dve_route(pt):
            """max(|u|,|v|) straight from PSUM: one reduce over the uv axis."""
            view = pt[:, 0:512].rearrange("p (uv px) -> p px uv", uv=2)
            dm = dpool.tile([128, 256], f16, tag="dm")
            nc.vector.tensor_reduce(dm[:], view, axis=mybir.AxisListType.X,
                                    op=mybir.AluOpType.max,
                                    apply_absolute_value=True)
            return clamp(dm)[:]

        pend = []

        def flush(limit):
            while len(pend) > limit:
                canvas_mm(*pend.pop(0))

        for j in ORDER:
            if j in dtile:
                pt, cs = dtile[j], 0
            else:
                pt, cs = pair_of[j]
            nc.tensor.matmul(pt[:, cs:cs + 256],
                             W_sb[:, 256 * j:256 * j + 128],
                             G_sb[:, F_SUB * j:F_SUB * (j + 1)],
                             start=True, stop=True)
            nc.tensor.matmul(pt[:, cs + 256:cs + 512],
                             W_sb[:, 256 * j + 128:256 * (j + 1)],
                             G_sb[:, F_SUB * j:F_SUB * (j + 1)],
                             start=True, stop=True)
            if j in (0, 1, NSG - 1):
                # Solo Abs chain.
                ab = abpool.tile([128, 512], f16, tag="ab")
                nc.scalar.activation(ab[:], pt[:],
                                     mybir.ActivationFunctionType.Abs)
                pend.append((j, max_min(ab, 0)))
            elif j == 5:
                # Pair-batched Abs over sgs 4 and 5, plus one batched
                # max-pair clamp: both dm halves land in one tile so a
                # single [128,512] min op covers both sgs.
                ab = abpool.tile([128, 1024], f16, tag="ab2")
                nc.scalar.activation(ab[:], pt[:],
                                     mybir.ActivationFunctionType.Abs)
                dm2 = dpool.tile([128, 512], f16, tag="dm2")
                nc.vector.tensor_tensor(dm2[:, 0:256], ab[:, 0:256],
                                        ab[:, 256:512],
                                        op=mybir.AluOpType.max)
                nc.vector.tensor_tensor(dm2[:, 256:512], ab[:, 512:768],
                                        ab[:, 768:1024],
                                        op=mybir.AluOpType.max)
                mr2 = rpool.tile([128, 512], f16, tag="mr2")
                nc.vector.tensor_scalar(mr2[:], dm2[:], 2.0, None,
                                        op0=mybir.AluOpType.min)
                pend.append((4, mr2[:, 0:256]))
                pend.append((5, mr2[:, 256:512]))
            elif j in ROUTE_D:
                pend.append((j, dve_route(pt)))
            flush(DELAY)
        flush(0)

    nc.compile()
    return nc


def _install_ntff_hook():
    """Provide antenv.axon_hooks if the image lacks it (ctypes shim around
    libaxon_pjrt.so's NRT profile capture). Returns True on success."""
    try:
        from antenv.axon_hooks import get_axon_ntff_profile_hook  # noqa: F401
        return True
    except ImportError:
        pass
    try:
        import contextlib
        import ctypes
        import types

        import antenv

        so_path = "/opt/axon/libaxon_pjrt.so"
        lib = ctypes.CDLL(so_path)
        if not hasattr(lib, "axon_start_nrt_profile"):
            return False
        lib.axon_start_nrt_profile.argtypes = [
            ctypes.POINTER(ctypes.c_int64), ctypes.c_size_t]
        lib.axon_start_nrt_profile.restype = ctypes.c_int64
        lib.axon_stop_nrt_profile.argtypes = [ctypes.c_char_p]
        lib.axon_stop_nrt_profile.restype = ctypes.c_int64

        @contextlib.contextmanager
        def _hook(output_dir, device_ids):
            import jax
            jax.devices()
            if device_ids:
                ids = (ctypes.c_int64 * len(device_ids))(*device_ids)
                rc = lib.axon_start_nrt_profile(ids, len(device_ids))
            else:
                rc = lib.axon_start_nrt_profile(None, 0)
            if rc != 0:
                raise RuntimeError(f"axon_start_nrt_profile rc={rc}")
            try:
                yield
            finally:
                n = lib.axon_stop_nrt_profile(str(output_dir).encode())
                print(f"ntff profile: {n} file(s) -> {output_dir}", file=sys.stderr)

        mod = types.ModuleType("antenv.axon_hooks")
        mod._hook = _hook
        mod.get_axon_ntff_profile_hook = lambda: _hook
        mod.set_axon_ntff_profile_hook = lambda h: None
        sys.modules["antenv.axon_hooks"] = mod
        antenv.axon_hooks = mod
        return True
    except Exception as e:  # pragma: no cover
        print("ntff hook install failed:", e, file=sys.stderr)
        return False


def _unshard(results, pack):
    """Reassemble per-core y [2, 128, 256] into the full (3, H, W)."""
    out = np.empty((3, H, W), np.float32)
    for core in range(N_CORES):
        y = np.asarray(results[core]["y"], np.float32)   # [2, 128, 256]
        for j, members in enumerate(pack[core]):
            bank, s = (0, j) if j < 4 else (1, j - 4)
            for ti, (band, cb, idx, off) in enumerate(members):
                blk = y[bank, 32 * s + 3 * ti:32 * s + 3 * ti + 3, :]
                out[:, band * SUB:(band + 1) * SUB,
                    cb * SUB:(cb + 1) * SUB] = blk.reshape(3, SUB, SUB)
    return out


def kernel(locations, matrix_offsets, matrix_scale_exponents, colors,
           canvas_height_px, canvas_width_px):
    assert int(canvas_height_px) == H and int(canvas_width_px) == W

    w_np, g_np, ct_np, b_np, pack = _prepare(
        locations, matrix_offsets, matrix_scale_exponents, colors)

    nc = _build_nc()

    from concourse.bass_utils import run_bass_kernel_spmd

    in_maps = [
        {"w": w_np[c], "g": g_np[c], "ct": ct_np[c], "b": b_np[c]}
        for c in range(N_CORES)
    ]
    trace = bool(int(os.environ.get("BASS_KERNEL_TRACE", "1")))
    if trace:
        trace = _install_ntff_hook()
    try:
        res = run_bass_kernel_spmd(nc, in_maps, core_ids=list(range(N_CORES)),
                                   trace=trace)
    except Exception:
        if not trace:
            raise
        res = run_bass_kernel_spmd(nc, in_maps, core_ids=list(range(N_CORES)),
                                   trace=False)
    last_run_info.clear()
    last_run_info.update(
        exec_time_ns=res.exec_time_ns,
        mean_exec_time_ns=res.mean_exec_time_ns,
        profile_json=res.profile_json,
    )

    return _unshard(res.results, pack)


# revision 48
# speedup vs baseline: 1.0578x; 1.0500x over previous
"""Trainium2 Bass kernel for the nn_Points problem.

Renders N=1024 anisotropic "diamond" points onto a 3x256x384 canvas:
    t = (pixel - loc) @ M_n          (2-vector per pixel per point)
    mapped = relu(1 - (|t0|+|t1|)/2)
    canvas = sigmoid(4 * sum_n mapped * color_n)

Design (8 NeuronCores, full inputs in / full output out):
  * u = t0+t1, v = t0-t1, so |t0|+|t1| = max(|u|,|v|) and
    mapped'' := min(max(|u|,|v|), 2) - 2 = -2*mapped (colors pre-scaled
    by -c/2 make the canvas matmul come out right).
  * The image is cut into 384 16x16 sub-tiles; an exact SAT cull (tile
    rect vs the preimage of the |t|_1<=2 diamond) lists each sub-tile's
    candidate points (mean ~17, max 32).  Sub-tiles are BIN-PACKED into
    "supergroups" of <= 128 candidate slots (variable per-tile counts -
    no fixed 32-slot padding), 7 supergroups per core, and the 8 cores'
    loads are balanced by splitting a band-major snake of sub-tiles at
    equal-candidate-count boundaries (not fixed 32-row slabs).
  * Per supergroup, TWO matmuls (u-coeffs / v-coeffs stationary [K,128],
    K<=40 shared-row layout: 3 fp16 hi/lo rows per distinct 16-row band,
    3 per distinct 16-col block, 2 const) share ONE moving operand
    [K, 256] (the 16x16 intra-tile pixel offsets) -> one PSUM bank
    [128, 512] = u | v on the SAME partitions.  max(|u|,|v|) therefore
    needs NO partition-realignment copy, and all 128 DVE lanes work.
  * Two chain flavors keep ACT and DVE both busy:
      ACT route: Abs [128,512] (one PSUM pass) -> DVE max fp16.
      DVE route: DMA copies the v half PSUM->SBUF; one DVE
        scalar_tensor_tensor (|u| abs_max |v|) reads u straight from
        PSUM.  (DVE may read one PSUM operand.)
    Then one DVE tensor_scalar min(d,2)-2 -> mr fp16 [128, 256].
  * One canvas matmul per supergroup (K=128 slots, stationary
    [128, 32] = 3g<=30 channel rows zero-padded, moving mr) into a
    2-bank canvas PSUM; sigmoid per bank (scale=4) + output DMA.
  * 3 dummy matmuls on a zeroed region run during the input-DMA wait so
    the PE's HAM activity window starts early (cold 1.2 GHz -> warm
    2.4 GHz after ~3.4us of sustained busy).
"""

import math
import os
import sys

import numpy as np

for _p in ("/opt/trn_rl_repo",):
    if _p not in sys.path and os.path.isdir(_p):
        sys.path.insert(0, _p)

# Geometry (matches the reference module's fixed canvas).
H, W = 256, 384
N_CORES = 8
SUB = 16                                # sub-tile edge (16x16)
N_BANDS = H // SUB                      # 16 row bands
N_CBLK = W // SUB                       # 24 col blocks
F_SUB = SUB * SUB                       # 256 moving columns per group
NSG = 7                                 # supergroups per core
KMAX = 40                               # contraction rows (shared layout)
GMAX = 10                               # sub-tiles per supergroup (3g <= 32)
WIDTH_TO_HEIGHT = 384.0 / 256.0

# Set BASS_KERNEL_TRACE=1 to capture an NTFF profile; results land here.
last_run_info = {}


def _hi_lo(x):
    """Split float64 array into fp16 hi + fp16 lo with tiny residual."""
    hi = x.astype(np.float16)
    lo = (x - hi.astype(np.float64)).astype(np.float16)
    return hi, lo


def _coeffs(locations, matrix_offsets, matrix_scale_exponents):
    loc = np.asarray(locations, np.float64).reshape(-1, 2)      # (N, 2) y,x
    mo = np.asarray(matrix_offsets, np.float64)                  # (N, 2, 2)
    mse = np.asarray(matrix_scale_exponents, np.float64).reshape(-1)
    n = loc.shape[0]

    scale = (math.sqrt(n) / 2.0) / np.exp(mse)
    mats = mo + np.eye(2)[None, :, :] * scale[:, None, None]     # (N, 2, 2)
    b = loc[:, 0, None] * mats[:, 0, :] + loc[:, 1, None] * mats[:, 1, :]

    wy_u = mats[:, 0, 0] + mats[:, 0, 1]
    wx_u = mats[:, 1, 0] + mats[:, 1, 1]
    c_u = -(b[:, 0] + b[:, 1])
    wy_v = mats[:, 0, 0] - mats[:, 0, 1]
    wx_v = mats[:, 1, 0] - mats[:, 1, 1]
    c_v = -(b[:, 0] - b[:, 1])
    return wy_u, wx_u, c_u, wy_v, wx_v, c_v


def _pack(locations, matrix_offsets, matrix_scale_exponents):
    """Cull candidates per 16x16 sub-tile, then bin-pack into supergroups.

    Returns per-core bins: core -> [ (members, K) ] where members is a
    list of (band, cblk, idx_array, slot_offset)."""
    wy_u, wx_u, c_u, wy_v, wx_v, c_v = _coeffs(
        locations, matrix_offsets, matrix_scale_exponents)

    # Preimage bbox of the |u|<=2,|v|<=2 diamond for the y/x SAT axes.
    det = wy_u * wx_v - wx_u * wy_v
    A00 = wx_v / det
    A01 = -wx_u / det
    A10 = -wy_v / det
    A11 = wy_u / det
    y0 = A00 * (-c_u) + A01 * (-c_v)
    x0 = A10 * (-c_u) + A11 * (-c_v)
    hy = 2 * (np.abs(A00) + np.abs(A01))
    hx = 2 * (np.abs(A10) + np.abs(A11))

    ys = np.linspace(-1.0, 1.0, H).astype(np.float32).astype(np.float64)
    xs = np.linspace(-WIDTH_TO_HEIGHT, WIDTH_TO_HEIGHT, W).astype(
        np.float32).astype(np.float64)

    def cull(r0, c0):
        ylo, yhi = ys[r0], ys[r0 + SUB - 1]
        xlo, xhi = xs[c0], xs[c0 + SUB - 1]
        yc, xc = (ylo + yhi) / 2, (xlo + xhi) / 2
        ry, rx = (yhi - ylo) / 2, (xhi - xlo) / 2
        ok = (np.abs(yc - y0) <= ry + hy + 1e-9) & \
             (np.abs(xc - x0) <= rx + hx + 1e-9)
        uc = wy_u * yc + wx_u * xc + c_u
        du = np.abs(wy_u) * ry + np.abs(wx_u) * rx
        ok &= np.abs(uc) <= 2 + du + 1e-9
        vc = wy_v * yc + wx_v * xc + c_v
        dv = np.abs(wy_v) * ry + np.abs(wx_v) * rx
        ok &= np.abs(vc) <= 2 + dv + 1e-9
        return np.nonzero(ok)[0]

    # Band-major snake over sub-tiles, cut into 8 equal-weight runs so the
    # per-core loads balance and each core spans few distinct bands.
    subs = []
    for band in range(N_BANDS):
        rng = range(N_CBLK) if band % 2 == 0 else range(N_CBLK - 1, -1, -1)
        for cb in rng:
            subs.append((band, cb, cull(band * SUB, cb * SUB)))
    msum = sum(len(s[2]) for s in subs)
    target = msum / N_CORES
    runs = [[] for _ in range(N_CORES)]
    acc = 0
    core = 0
    for s in subs:
        if core < N_CORES - 1 and acc + len(s[2]) / 2 > target * (core + 1):
            core += 1
        runs[core].append(s)
        acc += len(s[2])

    def pack_core(items):
        """Worst-fit decreasing into NSG bins under slot/size/K caps."""
        items = sorted(items, key=lambda s: -len(s[2]))
        bins = [{"m": 0, "items": [], "bands": set(), "cols": set()}
                for _ in range(NSG)]
        for s in items:
            m = len(s[2])
            placed = False
            for i in sorted(range(NSG), key=lambda i: bins[i]["m"]):
                bn = bins[i]
                nb = len(bn["bands"] | {s[0]})
                ncb = len(bn["cols"] | {s[1]})
                if (bn["m"] + m <= 128 and len(bn["items"]) < GMAX
                        and 3 * nb + 3 * ncb + 2 <= KMAX):
                    bn["m"] += m
                    bn["items"].append(s)
                    bn["bands"].add(s[0])
                    bn["cols"].add(s[1])
                    placed = True
                    break
            assert placed, "supergroup packing overflow"
        return bins

    cores = []
    for r in runs:
        bins = pack_core(r)
        core_bins = []
        for bn in bins:
            members = []
            off = 0
            for band, cb, idx in bn["items"]:
                members.append((band, cb, idx, off))
                off += len(idx)
            core_bins.append(members)
        cores.append(core_bins)
    return cores


def _prepare(locations, matrix_offsets, matrix_scale_exponents, colors):
    """Host-side prep: coefficients, packing, and device array fill."""
    wy_u, wx_u, c_u, wy_v, wx_v, c_v = _coeffs(
        locations, matrix_offsets, matrix_scale_exponents)
    cols = np.asarray(colors, np.float64).reshape(-1, 3)

    wyu_h, wyu_l = _hi_lo(wy_u)
    wxu_h, wxu_l = _hi_lo(wx_u)
    cu_h, cu_l = _hi_lo(c_u)
    wyv_h, wyv_l = _hi_lo(wy_v)
    wxv_h, wxv_l = _hi_lo(wx_v)
    cv_h, cv_l = _hi_lo(c_v)

    ys = np.linspace(-1.0, 1.0, H).astype(np.float32).astype(np.float64)
    xs = np.linspace(-WIDTH_TO_HEIGHT, WIDTH_TO_HEIGHT, W).astype(
        np.float32).astype(np.float64)
    gyh, gyl = _hi_lo(ys)
    gxh, gxl = _hi_lo(xs)

    pack = _pack(locations, matrix_offsets, matrix_scale_exponents)

    w_np = np.zeros((N_CORES, KMAX, NSG * 256), np.float16)
    g_np = np.zeros((N_CORES, KMAX, NSG * F_SUB), np.float16)
    ct_np = np.zeros((N_CORES, 128, NSG * 32), np.float16)

    for core in range(N_CORES):
        for j, members in enumerate(pack[core]):
            bands = sorted({b for b, _, _, _ in members})
            cblks = sorted({c for _, c, _, _ in members})
            yrow = {b: 3 * i for i, b in enumerate(bands)}
            xbase = 3 * len(bands)
            xrow = {c: xbase + 3 * i for i, c in enumerate(cblks)}
            crow = xbase + 3 * len(cblks)
            assert crow + 2 <= KMAX

            # Moving operand G [KMAX, 256]: per-band y rows (hi, lo, hi),
            # per-colblock x rows (hi, lo, hi), two const rows.
            go = j * F_SUB
            for b in bands:
                r = yrow[b]
                g_np[core, r + 0, go:go + F_SUB] = np.repeat(
                    gyh[b * SUB:(b + 1) * SUB], SUB)
                g_np[core, r + 1, go:go + F_SUB] = np.repeat(
                    gyl[b * SUB:(b + 1) * SUB], SUB)
                g_np[core, r + 2, go:go + F_SUB] = g_np[core, r + 0,
                                                        go:go + F_SUB]
            for c in cblks:
                r = xrow[c]
                g_np[core, r + 0, go:go + F_SUB] = np.tile(
                    gxh[c * SUB:(c + 1) * SUB], SUB)
                g_np[core, r + 1, go:go + F_SUB] = np.tile(
                    gxl[c * SUB:(c + 1) * SUB], SUB)
                g_np[core, r + 2, go:go + F_SUB] = g_np[core, r + 0,
                                                        go:go + F_SUB]
            g_np[core, crow, go:go + F_SUB] = 1.0
            g_np[core, crow + 1, go:go + F_SUB] = 1.0

            # Stationaries: u at cols 256j..+128, v at 256j+128..+128.
            uo = j * 256
            vo = j * 256 + 128
            for ti, (b, c, idx, off) in enumerate(members):
                m = len(idx)
                if m == 0:
                    continue
                sl = slice(off, off + m)
                for o, (wyh_, wyl_, wxh_, wxl_, ch_, cl_) in (
                        (uo, (wyu_h, wyu_l, wxu_h, wxu_l, cu_h, cu_l)),
                        (vo, (wyv_h, wyv_l, wxv_h, wxv_l, cv_h, cv_l))):
                    r = yrow[b]
                    w_np[core, r + 0, o + off:o + off + m] = wyh_[idx]
                    w_np[core, r + 1, o + off:o + off + m] = wyh_[idx]
                    w_np[core, r + 2, o + off:o + off + m] = wyl_[idx]
                    r = xrow[c]
                    w_np[core, r + 0, o + off:o + off + m] = wxh_[idx]
                    w_np[core, r + 1, o + off:o + off + m] = wxh_[idx]
                    w_np[core, r + 2, o + off:o + off + m] = wxl_[idx]
                    w_np[core, crow, o + off:o + off + m] = ch_[idx]
                    w_np[core, crow + 1, o + off:o + off + m] = cl_[idx]
                # Canvas stationary: slot rows off..off+m, channel cols.
                ct_np[core, sl, 32 * j + 3 * ti:32 * j + 3 * ti + 3] = (
                    -0.5 * cols[idx]).astype(np.float16)

    # Sigmoid bias folds the "-2" of mapped'' = min(d,2)-2:
    # sigma(4*(psum - 2*sum(ct))) -> bias[p] = -8*sum_slots ct[:, col(p)].
    b_np = np.zeros((N_CORES, 128, 2), np.float32)
    for core in range(N_CORES):
        colsum = ct_np[core].astype(np.float64).sum(axis=0)  # [NSG*32]
        for j in range(NSG):
            bank, s = (0, j) if j < 4 else (1, j - 4)
            b_np[core, 32 * s:32 * s + 32, bank] = (
                -8.0 * colsum[32 * j:32 * j + 32])

    return w_np, g_np, ct_np, b_np, pack


def _build_nc():
    """Build the Bass/Tile program (shared by all cores)."""
    from contextlib import ExitStack

    import concourse.bacc as bacc
    import concourse.tile as tile
    from concourse import mybir

    f16 = mybir.dt.float16
    f32 = mybir.dt.float32
    nc = bacc.Bacc("TRN2", target_bir_lowering=False, debug=False,
                   num_devices=N_CORES)

    w_d = nc.dram_tensor("w", [KMAX, NSG * 256], f16, kind="ExternalInput")
    g_d = nc.dram_tensor("g", [KMAX, NSG * F_SUB], f16, kind="ExternalInput")
    ct_d = nc.dram_tensor("ct", [128, NSG * 32], f16, kind="ExternalInput")
    b_d = nc.dram_tensor("b", [128, 2], f32, kind="ExternalInput")
    # y[bank, slot_partition, px]
    y_d = nc.dram_tensor("y", [2, 128, F_SUB], f16, kind="ExternalOutput")

    ROUTE_D = {2, 3}         # all-DVE route (reads PSUM at fp32 rate);
    # {0,1} get solo ACT Abs passes, {4,5} a pair-batched one, {6} a solo
    # Abs (late in the kernel ACT frees up first while DVE has backlog).
    ORDER = [0, 1, 2, 3, 4, 5, 6]
    DELAY = 4                # canvas matmuls trail the uv stream

    with ExitStack() as ctx:
        tc = ctx.enter_context(tile.TileContext(nc))
        const = ctx.enter_context(tc.tile_pool(name="const", bufs=1))
        uvpool = ctx.enter_context(tc.tile_pool(name="uv", bufs=1,
                                                space="PSUM"))
        cvpool = ctx.enter_context(tc.tile_pool(name="cv", bufs=1,
                                                space="PSUM"))
        abpool = ctx.enter_context(tc.tile_pool(name="ab", bufs=2))
        vpool = ctx.enter_context(tc.tile_pool(name="vs", bufs=3))
        dpool = ctx.enter_context(tc.tile_pool(name="dm", bufs=3))
        rpool = ctx.enter_context(tc.tile_pool(name="mr", bufs=6))
        opool = ctx.enter_context(tc.tile_pool(name="o", bufs=1))

        W_sb = const.tile([KMAX, NSG * 256], f16)
        G_sb = const.tile([KMAX, NSG * F_SUB], f16)
        CT_sb = const.tile([128, NSG * 32], f16)
        B_sb = const.tile([128, 2], f32)
        # Tiny zeroed region feeding the act-table-pinning sigmoid.
        # (PE warmup matmuls were tried and dropped: on this part the HAM
        # clock-gate never releases - matmuls run at 1.2 GHz throughout -
        # so warmups only delayed the real stream.)
        warm = const.tile([128, 16], f16)

        # Input DMA split: the scalar queue boots straight into a DMA (no
        # preamble memsets like gpsimd), so the g-lead lands earliest
        # there - it would otherwise gate the first matmul; the act-table
        # load queues right behind it, still well before the first Abs.
        # The memset for the table-pinning sigmoid goes on the (otherwise
        # idle until mid-kernel) vector engine.
        nc.vector.memset(warm[:], 0.0)
        nc.scalar.dma_start(G_sb[:, 0:256], g_d[:, 0:256])
        nc.sync.dma_start(W_sb[:, 0:256], w_d[:, 0:256])
        nc.gpsimd.dma_start(G_sb[:, 256:1024], g_d[:, 256:1024])
        nc.sync.dma_start(W_sb[:, 256:1024], w_d[:, 256:1024])
        nc.sync.dma_start(W_sb[:, 1024:1792], w_d[:, 1024:1792])
        nc.gpsimd.dma_start(G_sb[:, 1024:1792], g_d[:, 1024:1792])
        nc.gpsimd.dma_start(CT_sb[:], ct_d[:])
        nc.gpsimd.dma_start(B_sb[:], b_d[:])

        # Canvas PSUM: bank 0 = supergroups 0-3, bank 1 = 4-6.
        cv0 = cvpool.tile([128, 512], f32, tag="c0", bufs=1)
        cv1 = cvpool.tile([128, 512], f32, tag="c1", bufs=1)
        outr0 = opool.tile([128, F_SUB], f16, tag="o0", bufs=1)
        outr1 = opool.tile([128, F_SUB], f16, tag="o1", bufs=1)

        # Pin the act-table set that holds BOTH Abs and Sigmoid by issuing
        # a tiny sigmoid first; Abs and the final sigmoids then share one
        # table and no mid-kernel ACT_TABLE_LOAD lands on the chain.
        warmo = opool.tile([128, 1], f32, tag="wo", bufs=1)
        nc.scalar.activation(warmo[:], warm[:, 0:1],
                             mybir.ActivationFunctionType.Sigmoid)

        # u|v PSUM layout (8 banks exactly, with cv0/cv1):
        #  - sgs 0 and 1: own 1-bank tiles with SOLO Abs passes.  Early in
        #    the kernel ACT has slack, and a solo Abs right after sg0's
        #    matmuls starts the DVE chain ~0.7us earlier than a pair-
        #    batched one (which must wait for sg1's matmuls too).
        #  - sgs 4,5: one 2-bank tile, pair-batched Abs (mid-kernel ACT is
        #    the busy engine, so batching saves its per-op overhead).
        #  - sgs {2,3} (DVE reduce route) and 6 (solo Abs): a rotating
        #    2x1-bank pool.
        p0 = uvpool.tile([128, 512], f32, tag="p0", bufs=1)
        p1 = uvpool.tile([128, 512], f32, tag="p1", bufs=1)
        pairB = uvpool.tile([128, 1024], f32, tag="pb", bufs=1)
        pair_of = {0: (p0, 0), 1: (p1, 0), 4: (pairB, 0), 5: (pairB, 512)}
        # sg 6 (solo Abs) reuses sg 0's bank - free once sg 0's Abs read it.
        dtile = {NSG - 1: p0}
        for j in (2, 3):
            dtile[j] = uvpool.tile([128, 512], f32, tag="pd", bufs=2,
                                   name=f"pd{j}")

        bank_left = [4, 3]

        def canvas_mm(j, mr):
            if j < 4:
                cv, s = cv0, j
            else:
                cv, s = cv1, j - 4
            # Deprioritized so the tile scheduler never lets a canvas
            # matmul preempt the uv stream (the PE is free afterwards,
            # while a delayed uv matmul delays its whole chain).
            with tc.high_priority(offset=-100000):
                nc.tensor.matmul(cv[32 * s:32 * s + 32, 0:F_SUB],
                                 CT_sb[:, 32 * j:32 * j + 32], mr,
                                 start=True, stop=True,
                                 tile_position=(0, 32 * s))
            # canvas holds sum(ct * min(d,2)); the "-2" term is folded into
            # the per-partition sigmoid bias (bias = -8*sum(ct)).
            bank = 0 if j < 4 else 1
            bank_left[bank] -= 1
            if bank_left[bank]:
                return
            if bank == 0:
                nc.scalar.activation(outr0[:], cv0[:, 0:F_SUB],
                                     mybir.ActivationFunctionType.Sigmoid,
                                     bias=B_sb[:, 0:1], scale=4.0)
                nc.sync.dma_start(y_d[0], outr0[:])
            else:
                nc.scalar.activation(outr1[0:96, :], cv1[0:96, 0:F_SUB],
                                     mybir.ActivationFunctionType.Sigmoid,
                                     bias=B_sb[0:96, 1:2], scale=4.0)
                # Final output: split across the sync and scalar queues -
                # this DMA is pure tail latency (scalar is free after the
                # last sigmoid; gpsimd sees the sigmoid's semaphore ~0.4us
                # later than the others, so it gets no slice).
                nc.sync.dma_start(y_d[1, 0:48], outr1[0:48, :])
                nc.scalar.dma_start(y_d[1, 48:96], outr1[48:96, :])

        def clamp(dm):
            """min(d,2). (gpsimd was tried for this and is ~20x slower -
            its tensor ops are software DSP loops - so it stays on DVE.)"""
            mr = rpool.tile([128, 256], f16, tag="mr")
            nc.vector.tensor_scalar(mr[:], dm[:], 2.0, None,
                                    op0=mybir.AluOpType.min)
            return mr

        def max_min(ab, o):
            """fp16 max over the u|v column halves, then clamp at 2."""
            dm = dpool.tile([128, 256], f16, tag="dm")
            nc.vector.tensor_tensor(dm[:], ab[:, o:o + 256],
                                    ab[:, o + 256:o + 512],
                                    op=mybir.AluOpType.max)
            return clamp(dm)[:]

        def dve_route(pt):
            """max(|u|,|v|) straight from PSUM: one reduce over the uv axis."""
            view = pt[:, 0:512].rearrange("p (uv px) -> p px uv", uv=2)
            dm = dpool.tile([128, 256], f16, tag="dm")
            nc.vector.tensor_reduce(dm[:], view, axis=mybir.AxisListType.X,
                                    op=mybir.AluOpType.max,
                                    apply_absolute_value=True)
            return clamp(dm)[:]

        pend = []

        def flush(limit):
            while len(pend) > limit:
                canvas_mm(*pend.pop(0))

        for j in ORDER:
            if j in dtile:
                pt, cs = dtile[j], 0
            else:
                pt, cs = pair_of[j]
            nc.tensor.matmul(pt[:, cs:cs + 256],
                             W_sb[:, 256 * j:256 * j + 128],
                             G_sb[:, F_SUB * j:F_SUB * (j + 1)],
                             start=True, stop=True)
            nc.tensor.matmul(pt[:, cs + 256:cs + 512],
                             W_sb[:, 256 * j + 128:256 * (j + 1)],
                             G_sb[:, F_SUB * j:F_SUB * (j + 1)],
                             start=True, stop=True)
            if j in (0, 1, NSG - 1):
                # Solo Abs chain.
                ab = abpool.tile([128, 512], f16, tag="ab")
                nc.scalar.activation(ab[:], pt[:],
                                     mybir.ActivationFunctionType.Abs)
                pend.append((j, max_min(ab, 0)))
            elif j == 5:
                # Pair-batched Abs over sgs 4 and 5, plus one batched
                # max-pair clamp: both dm halves land in one tile so a
                # single [128,512] min op covers both sgs.
                ab = abpool.tile([128, 1024], f16, tag="ab2")
                nc.scalar.activation(ab[:], pt[:],
                                     mybir.ActivationFunctionType.Abs)
                dm2 = dpool.tile([128, 512], f16, tag="dm2")
                nc.vector.tensor_tensor(dm2[:, 0:256], ab[:, 0:256],
                                        ab[:, 256:512],
                                        op=mybir.AluOpType.max)
                nc.vector.tensor_tensor(dm2[:, 256:512], ab[:, 512:768],
                                        ab[:, 768:1024],
                                        op=mybir.AluOpType.max)
                mr2 = rpool.tile([128, 512], f16, tag="mr2")
                nc.vector.tensor_scalar(mr2[:], dm2[:], 2.0, None,
                                        op0=mybir.AluOpType.min)
                pend.append((4, mr2[:, 0:256]))
                pend.append((5, mr2[:, 256:512]))
            elif j in ROUTE_D:
                pend.append((j, dve_route(pt)))
            flush(DELAY)
        flush(0)

    nc.compile()
    return nc


def _install_ntff_hook():
    """Provide antenv.axon_hooks if the image lacks it (ctypes shim around
    libaxon_pjrt.so's NRT profile capture). Returns True on success."""
    try:
        from antenv.axon_hooks import get_axon_ntff_profile_hook  # noqa: F401
        return True
    except ImportError:
        pass
    try:
        import contextlib
        import ctypes
        import types

        import antenv

        so_path = "/opt/axon/libaxon_pjrt.so"
        lib = ctypes.CDLL(so_path)
        if not hasattr(lib, "axon_start_nrt_profile"):
            return False
        lib.axon_start_nrt_profile.argtypes = [
            ctypes.POINTER(ctypes.c_int64), ctypes.c_size_t]
        lib.axon_start_nrt_profile.restype = ctypes.c_int64
        lib.axon_stop_nrt_profile.argtypes = [ctypes.c_char_p]
        lib.axon_stop_nrt_profile.restype = ctypes.c_int64

        @contextlib.contextmanager
        def _hook(output_dir, device_ids):
            import jax
            jax.devices()
            if device_ids:
                ids = (ctypes.c_int64 * len(device_ids))(*device_ids)
                rc = lib.axon_start_nrt_profile(ids, len(device_ids))
            else:
                rc = lib.axon_start_nrt_profile(None, 0)
            if rc != 0:
                raise RuntimeError(f"axon_start_nrt_profile rc={rc}")
            try:
                yield
            finally:
                n = lib.axon_stop_nrt_profile(str(output_dir).encode())
                print(f"ntff profile: {n} file(s) -> {output_dir}", file=sys.stderr)

        mod = types.ModuleType("antenv.axon_hooks")
        mod._hook = _hook
        mod.get_axon_ntff_profile_hook = lambda: _hook
        mod.set_axon_ntff_profile_hook = lambda h: None
        sys.modules["antenv.axon_hooks"] = mod
        antenv.axon_hooks = mod
        return True
    except Exception as e:  # pragma: no cover
        print("ntff hook install failed:", e, file=sys.stderr)
        return False


def _unshard(results, pack):
    """Reassemble per-core y [2, 128, 256] into the full (3, H, W)."""
    out = np.empty((3, H, W), np.float32)
    for core in range(N_CORES):
        y = np.asarray(results[core]["y"], np.float32)   # [2, 128, 256]
        for j, members in enumerate(pack[core]):
            bank, s = (0, j) if j < 4 else (1, j - 4)
            for ti, (band, cb, idx, off) in enumerate(members):
                blk = y[bank, 32 * s + 3 * ti:32 * s + 3 * ti + 3, :]
                out[:, band * SUB:(band + 1) * SUB,
                    cb * SUB:(cb + 1) * SUB] = blk.reshape(3, SUB, SUB)
    return out


def kernel(locations, matrix_offsets, matrix_scale_exponents, colors,
           canvas_height_px, canvas_width_px):
    assert int(canvas_height_px) == H and int(canvas_width_px) == W

    w_np, g_np, ct_np, b_np, pack = _prepare(
        locations, matrix_offsets, matrix_scale_exponents, colors)

    nc = _build_nc()

    from concourse.bass_utils import run_bass_kernel_spmd

    in_maps = [
        {"w": w_np[c], "g": g_np[c], "ct": ct_np[c], "b": b_np[c]}
        for c in range(N_CORES)
    ]
    trace = bool(int(os.environ.get("BASS_KERNEL_TRACE", "1")))
    if trace:
        trace = _install_ntff_hook()
    try:
        res = run_bass_kernel_spmd(nc, in_maps, core_ids=list(range(N_CORES)),
                                   trace=trace)
    except Exception:
        if not trace:
            raise
        res = run_bass_kernel_spmd(nc, in_maps, core_ids=list(range(N_CORES)),
                                   trace=False)
    last_run_info.clear()
    last_run_info.update(
        exec_time_ns=res.exec_time_ns,
        mean_exec_time_ns=res.mean_exec_time_ns,
        profile_json=res.profile_json,
    )

    return _unshard(res.results, pack)
